# revision 1
# baseline (speedup 1.0000x reference)
"""Causal self-attention (B=4, T=2048, E=1024, H=16, D=64) on 8 TRN2 NeuronCores.

Sharding: data-parallel over batch (4) x tensor-parallel over heads (2 groups
of 8).  Core c handles batch b=c//2, head group g=c%2.

Per-core pipeline (all matmuls bf16 on TensorE, fp32 PSUM accumulation):
  A) qkv projection from pre-transposed x^T: q^T,k^T in [feat, tok] layout,
     v in natural [tok, feat] layout with a ones column per head (so the av
     matmul's 65th output row accumulates the softmax denominator Z).
  B) per head, per 1024-wide q window: scores^T = k^T_blk.T @ q^T into a
     2-bank PSUM tile -> one wide exp per k-block (ScalarE, scale=1/8, no
     max-subtraction: |scores|<4 for this data; causally-dead columns are
     trimmed, the diagonal 128x128 gets a triangular mask multiply on DVE)
     -> y^T[65, q] accumulation with v_aug -> normalize by 1/Z (DVE
     reciprocal + GpSimd partition_broadcast + DVE multiply).
     ScalarE's exp is the stage-B critical path, so the next pair's q/k
     projection matmuls are interleaved into the k-block loop as PE filler.
  C) output projection partials + const/2 (both pair cores add half, so the
     ReduceScatter sum restores the full constant) -> 4 chunked
     ReduceScatters over the neighbor pair, each DMA'd DRAM->DRAM straight
     to the output: chunk k reduces out-feat blocks {2k, 2k+1}; the pair's
     even core receives block 2k, the odd core 2k+1 (host reassembles).

Bias algebra: k bias is softmax-shift-invariant (dropped); v bias commutes
with the (row-stochastic) attention weights so it is folded with proj_b
into the output constant on the host; q bias is applied on-device.
"""

import sys

if "/opt/trn_rl_repo" not in sys.path:
    sys.path.insert(0, "/opt/trn_rl_repo")

import ml_dtypes
import numpy as np

import concourse.bass as bass
import concourse.mybir as mybir
import concourse.tile as tile
from concourse import bacc
from concourse.bass_utils import run_bass_kernel_spmd

B, T, E = 4, 2048, 1024
H, D = 16, 64
N_CORES = 8
F = 512          # local features per core (8 heads * 64)
HPC = 8          # heads per core
EC = E // 128    # 8 emb chunks
TC = T // 512    # 4 token chunks of 512
TB = T // 128    # 16 token blocks of 128
FB = F // 128    # 4 local feature blocks
OB = E // 128    # 8 output feature blocks
SCALE = 0.125    # 1/sqrt(D)

BF16 = mybir.dt.bfloat16
F32 = mybir.dt.float32
_nbf16 = ml_dtypes.bfloat16

_CACHED_NC = None


def build_nc(repeat=1, single_core=False):
    nc = bacc.Bacc("TRN2", target_bir_lowering=False, debug=False,
                   num_devices=1 if single_core else N_CORES)

    xT = nc.declare_dram_parameter("xT", [E, T], BF16, isOutput=False)
    wqT = nc.declare_dram_parameter("wqT", [E, F], BF16, isOutput=False)
    wkT = nc.declare_dram_parameter("wkT", [E, F], BF16, isOutput=False)
    wvT = nc.declare_dram_parameter("wvT", [E, F], BF16, isOutput=False)
    pwT = nc.declare_dram_parameter("pwT", [F, E], BF16, isOutput=False)
    bqd = nc.declare_dram_parameter("bq", [128, FB], F32, isOutput=False)
    cvd = nc.declare_dram_parameter("constv", [128, OB], F32, isOutput=False)
    out = nc.declare_dram_parameter("out", [F, T], F32, isOutput=True)

    AF = mybir.ActivationFunctionType
    ALU = mybir.AluOpType

    with tile.TileContext(nc) as tc:
        with (
            tc.tile_pool(name="persist", bufs=1) as pers,
            tc.tile_pool(name="work", bufs=6) as work,
            tc.tile_pool(name="evac", bufs=3) as evac,
            tc.tile_pool(name="psP", bufs=3, space="PSUM") as psP,
            tc.tile_pool(name="dram", bufs=1, space="DRAM") as dram,
        ):
            # ---- constants ----
            bq_t = pers.tile([128, FB], F32, tag="bq")
            cv_t = pers.tile([128, OB], F32, tag="cv")
            nc.sync.dma_start(bq_t[:], bqd[:])
            nc.sync.dma_start(cv_t[:], cvd[:])

            # upper-triangular (incl diag) ones [128, 128] bf16 for the
            # in-window diagonal block mask
            tri = pers.tile([128, 128], BF16, tag="tri")
            nc.gpsimd.memset(tri[:], 0.0)
            nc.gpsimd.affine_select(
                out=tri[:], in_=tri[:],
                compare_op=ALU.is_gt, fill=1.0,
                base=0, pattern=[[-1, 128]], channel_multiplier=1,
            )

            for _rep in range(repeat):
                # ---- persistent activations / weights ----
                xt = [pers.tile([128, T], BF16, tag=f"xT{ec}", name=f"xT{ec}") for ec in range(EC)]
                wq = [pers.tile([128, F], BF16, tag=f"wq{ec}", name=f"wq{ec}") for ec in range(EC)]
                wk = [pers.tile([128, F], BF16, tag=f"wk{ec}", name=f"wk{ec}") for ec in range(EC)]
                wv = [pers.tile([128, F], BF16, tag=f"wv{ec}", name=f"wv{ec}") for ec in range(EC)]
                pw = [pers.tile([128, E], BF16, tag=f"pw{fc}", name=f"pw{fc}") for fc in range(FB)]
                for ec in range(EC):
                    sl = slice(ec * 128, (ec + 1) * 128)
                    nc.sync.dma_start(xt[ec][:, 0:1024], xT[sl, 0:1024])
                    nc.sync.dma_start(wq[ec][:], wqT[sl, :])
                for ec in range(EC):
                    sl = slice(ec * 128, (ec + 1) * 128)
                    nc.sync.dma_start(wk[ec][:], wkT[sl, :])
                    nc.sync.dma_start(wv[ec][:], wvT[sl, :])
                for ec in range(EC):
                    sl = slice(ec * 128, (ec + 1) * 128)
                    nc.sync.dma_start(xt[ec][:, 1024:T], xT[sl, 1024:T])
                for fc in range(FB):
                    nc.sync.dma_start(pw[fc][:], pwT[fc * 128:(fc + 1) * 128, :])

                qT = [pers.tile([128, T], BF16, tag=f"qT{fb}", name=f"qT{fb}") for fb in range(FB)]
                kT = [pers.tile([128, T], BF16, tag=f"kT{fb}", name=f"kT{fb}") for fb in range(FB)]
                # v natural layout with per-head ones column: [vh(64) | 1] * 8
                va = [pers.tile([128, 520], BF16, tag=f"va{tb}", name=f"va{tb}") for tb in range(TB)]
                yT = [pers.tile([128, T], BF16, tag=f"yT{fb}", name=f"yT{fb}") for fb in range(FB)]

                # ---- stage A: qkv projections ([128,1024] psum windows) ----
                def qk_unit(fb, w2, which):
                    fsl = slice(fb * 128, (fb + 1) * 128)
                    wgt, dst, bias = ((wq, qT, True) if which == "q"
                                      else (wk, kT, False))
                    ps = psP.tile([128, 1024], F32, tag="big", name="psA")
                    for half in range(2):
                        tsl = slice(w2 * 1024 + half * 512,
                                    w2 * 1024 + (half + 1) * 512)
                        psl = slice(half * 512, (half + 1) * 512)
                        for ec in range(EC):
                            nc.tensor.matmul(
                                ps[:, psl], wgt[ec][:, fsl], xt[ec][:, tsl],
                                start=(ec == 0), stop=(ec == EC - 1))
                    wsl = slice(w2 * 1024, (w2 + 1) * 1024)
                    if bias:
                        nc.vector.tensor_scalar_add(dst[fb][:, wsl], ps[:],
                                                    bq_t[:, fb:fb + 1])
                    else:
                        nc.vector.tensor_copy(dst[fb][:, wsl], ps[:])

                def qk_proj(fb):
                    for w2 in range(T // 1024):
                        qk_unit(fb, w2, "q")
                        qk_unit(fb, w2, "k")

                def v_proj(tb):
                    bsl = slice(tb * 128, (tb + 1) * 128)
                    ps = psP.tile([128, 512], F32, tag="big", name="psV")
                    for ec in range(EC):
                        nc.tensor.matmul(ps[:], xt[ec][:, bsl], wv[ec][:],
                                         start=(ec == 0), stop=(ec == EC - 1))
                    nc.gpsimd.memset(va[tb][:], 1.0)
                    # one strided copy: [128, 8x64] -> cols {65h..65h+63}
                    nc.vector.tensor_copy(
                        va[tb].rearrange("p (h c) -> p h c", h=HPC)[:, :, 0:64],
                        ps[:].rearrange("p (h c) -> p h c", h=HPC))


                # ---- stage B: attention, 1024-wide q windows; PE filler
                # work (next pair's q/k projection) is injected between kb
                # blocks so the PE keeps producing while ACT drains exps ----
                fillers = []

                def normalize(h, qc, psy_t):
                    fb, po = h // 2, (h % 2) * 64
                    qsl = slice(qc * 512, (qc + 1) * 512)
                    # yT = psy[0:64] * (1/Z): row-broadcast 1/Z on Pool
                    rz = evac.tile([1, 512], BF16, tag="rz")
                    with nc.allow_low_precision(
                            reason="1/Z in bf16; Z is O(1e2), "
                            "0.4% relative is within budget"):
                        nc.vector.reciprocal(rz[:], psy_t[64:65, :])
                    zb = evac.tile([64, 512], BF16, tag="zb")
                    nc.gpsimd.partition_broadcast(zb[:], rz[:])
                    nc.vector.tensor_mul(yT[fb][po:po + 64, qsl],
                                         psy_t[0:64, :], zb[:])

                def attn_win(h, w, jit_v=False):
                    fb, po = h // 2, (h % 2) * 64
                    qh = qT[fb][po:po + 64, :]
                    kh = kT[fb][po:po + 64, :]
                    if True:
                        psy = {}
                        for qc in (2 * w, 2 * w + 1):
                            psy[qc] = psP.tile([65, 512], F32, tag="psy",
                                               bufs=2, name="psy")
                        for kb in range(8 * w + 8):
                            if jit_v and kb + 1 < TB // 2:
                                # first window: emit v blocks just ahead of
                                # their av consumers instead of all upfront
                                v_proj(kb + 1)
                            if kb == 8 * w + 4:
                                # even-qc psum complete: normalize now so its
                                # bank frees mid-window
                                normalize(h, 2 * w, psy[2 * w])
                            if fillers:
                                fillers.pop(0)()
                            j = kb - 8 * w
                            off = max(j, 0) * 128
                            pss = psP.tile([128, 1024], F32, tag="big",
                                           name="pss")
                            ksl = slice(kb * 128, (kb + 1) * 128)
                            for half in range(2):
                                lo = max(off, half * 512)
                                hi = (half + 1) * 512
                                if lo >= hi:
                                    continue
                                nc.tensor.matmul(
                                    pss[:, lo:hi], kh[:, ksl],
                                    qh[:, w * 1024 + lo:w * 1024 + hi],
                                    start=True, stop=True)
                            at = work.tile([128, 1024], BF16, tag="attT")
                            nc.scalar.activation(at[:, off:1024], pss[:, off:1024],
                                                 AF.Exp, scale=SCALE)
                            if j >= 0:
                                nc.vector.tensor_mul(at[:, off:off + 128],
                                                     at[:, off:off + 128], tri[:])
                            for halfq in range(2):
                                qc = 2 * w + halfq
                                if kb > 4 * qc + 3:
                                    continue
                                lo = max(off, halfq * 512)
                                hi = (halfq + 1) * 512
                                nc.tensor.matmul(
                                    psy[qc][:, lo - halfq * 512:hi - halfq * 512],
                                    va[kb][:, h * 65:h * 65 + 65], at[:, lo:hi],
                                    start=(kb == 0), stop=(kb == 4 * qc + 3))
                        normalize(h, 2 * w + 1, psy[2 * w + 1])

                qk_unit(0, 0, "q")
                qk_unit(0, 0, "k")
                v_proj(0)
                fillers.append(lambda: qk_unit(0, 1, "q"))
                fillers.append(lambda: qk_unit(0, 1, "k"))
                vfill = [(lambda tb=tb: v_proj(tb)) for tb in range(8, TB)]
                fillers.extend(vfill)
                fillers.extend(
                    (lambda f=f, w2=w2, wh=wh: qk_unit(f, w2, wh))
                    for f in range(1, FB)
                    for w2 in range(T // 1024) for wh in ("q", "k"))
                for pair in range(FB):
                    attn_win(2 * pair, 0, jit_v=(pair == 0))
                    attn_win(2 * pair + 1, 0)
                    if pair == 0:
                        # emission-order dependency: va[8..15] and qk0's
                        # second window must be emitted before w=1 consumes
                        while any(f in vfill for f in fillers):
                            fillers.pop(0)()
                    attn_win(2 * pair, 1)
                    attn_win(2 * pair + 1, 1)
                while fillers:
                    fillers.pop(0)()

                # ---- stage C: projection + chunked ReduceScatter ----
                # chunk k reduces out-feat blocks {2k, 2k+1}; the pair's even
                # core receives block 2k, the odd core block 2k+1 (host
                # reassembles).  Chunking overlaps RS/final with later proj.
                for ck in range(OB // 2):
                    yTp = dram.tile([256, T], F32, tag="yTp", name=f"yTp{ck}")
                    yTr = dram.tile([128, T], F32, tag="yTr", name=f"yTr{ck}")
                    for obh in range(2):
                        ob = 2 * ck + obh
                        osl = slice(ob * 128, (ob + 1) * 128)
                        for w2 in range(T // 1024):
                            ps = psP.tile([128, 1024], F32, tag="big",
                                          name="psC")
                            for half in range(2):
                                tsl = slice(w2 * 1024 + half * 512,
                                            w2 * 1024 + (half + 1) * 512)
                                psl = slice(half * 512, (half + 1) * 512)
                                for fc in range(FB):
                                    nc.tensor.matmul(ps[:, psl], pw[fc][:, osl],
                                                     yT[fc][:, tsl],
                                                     start=(fc == 0),
                                                     stop=(fc == FB - 1))
                            st = evac.tile([128, 1024], F32, tag="pjevac")
                            # + const/2 here: both pair cores add half, the
                            # ReduceScatter sum restores the full constant
                            nc.vector.tensor_scalar_add(st[:], ps[:],
                                                        cv_t[:, ob:ob + 1])
                            nc.sync.dma_start(
                                yTp[obh * 128:(obh + 1) * 128,
                                    w2 * 1024:(w2 + 1) * 1024], st[:])
                    if single_core:
                        # timeline-sim stand-in for the pair ReduceScatter
                        nc.sync.dma_start(yTr[:], yTp[0:128, :])
                    else:
                        nc.gpsimd.collective_compute(
                            "ReduceScatter",
                            ALU.add,
                            replica_groups=[[0, 1], [2, 3], [4, 5], [6, 7]],
                            ins=[yTp.opt()],
                            outs=[yTr.opt()],
                        )
                    nc.sync.dma_start(out[ck * 128:(ck + 1) * 128, :], yTr[:])

    nc.compile()
    return nc


def _get_nc():
    global _CACHED_NC
    if _CACHED_NC is None:
        _CACHED_NC = build_nc()
    return _CACHED_NC


def make_in_maps(x, qkv_w, qkv_b, proj_w, proj_b):
    x = np.asarray(x, np.float32)
    qkv_w = np.asarray(qkv_w, np.float32)
    qkv_b = np.asarray(qkv_b, np.float32)
    proj_w = np.asarray(proj_w, np.float32)
    proj_b = np.asarray(proj_b, np.float32)

    const = proj_b + proj_w @ qkv_b[2 * E:3 * E]  # v-bias folded through proj
    in_maps = []
    for c in range(N_CORES):
        b, g = c // 2, c % 2
        gsl = slice(g * F, (g + 1) * F)
        xTb = np.ascontiguousarray(x[b].T).astype(_nbf16)
        m = {
            "xT": xTb,
            "wqT": np.ascontiguousarray(qkv_w[gsl].T).astype(_nbf16),
            "wkT": np.ascontiguousarray(qkv_w[E + g * F:E + (g + 1) * F].T
                                        ).astype(_nbf16),
            "wvT": np.ascontiguousarray(qkv_w[2 * E + g * F:2 * E + (g + 1) * F].T
                                        ).astype(_nbf16),
            "pwT": np.ascontiguousarray(proj_w[:, gsl].T).astype(_nbf16),
            "bq": np.ascontiguousarray(qkv_b[gsl].reshape(FB, 128).T
                                       ).astype(np.float32),
            # const/2 is added pre-ReduceScatter by both pair cores;
            # col ob = const[ob block] / 2
            "constv": np.ascontiguousarray(
                const.reshape(OB, 128).T / 2.0).astype(np.float32),
        }
        in_maps.append(m)
    return in_maps


def assemble_output(results):
    y = np.empty((B, T, E), np.float32)
    for c in range(N_CORES):
        b, g = c // 2, c % 2
        o = results[c]["out"]  # [512, T]: row block k = out-feat block 2k+g
        for k in range(FB):
            blk = 2 * k + g
            y[b][:, blk * 128:(blk + 1) * 128] = o[k * 128:(k + 1) * 128].T
    return y


def kernel(**inputs):
    nc = _get_nc()
    in_maps = make_in_maps(**inputs)
    res = run_bass_kernel_spmd(nc, in_maps, list(range(N_CORES)))
    return assemble_output(res.results)

